# revision 70
# baseline (speedup 1.0000x reference)
"""AlternatingDiffHead Trainium2 kernel.

Data-parallel over batch: B=8 batch elements -> 8 NeuronCores, one batch
element per core, no collectives.

Per-core math (T=2048, C=1024, HS=128, 2 terms):
  v  = x @ Wv                                  [T, 256]
  qn = rope(x @ Wqn * 1/sqrt(HS)),  kn = rope(x @ Wkn)     [T, 128]
  Sn = qn @ kn^T  (causal)                      [T, T]
  En = exp(Sn)    (no max-sub; S is O(1))       rowsum -> ln
  D  = E0 + beta E1,  beta[t] = (c1 l0[t]) / (c0 l1[t])
  out[t] = (c0 / l0[t]) * (D @ v)[t]
where c0 = lam0, c1 = -lam1 (host-computed scalars).

v6 design notes (on top of v3):
 - Inputs interleaved (w chunk c, x chunk-half c) across both hw queues
   in consumption order, x in g-halves matching the projection loop, so
   the first matmul starts ~14us in and is never starved.
 - RoPE add for group g emitted during group g+1's matmuls so the
   partition-swap DMA latency never blocks the DVE FIFO (this gated the
   whole scores phase by ~15us).  Swaps ride the idle hw queues.
 - Three phases: (1) g=0 projections; (2) g=1 projections overlapped
   with v-projs 0..7 and the complete score->exp->D->D^T chains of the
   eight small row-tiles (W<=1024 needs only the g=0 q/k halves) --
   ACT/DVE/sync work there instead of idling until all projections
   finish; (3) big tiles descending [15..8] with the small AVs as early
   PE filler, AV(i) four iterations behind scores(i), D-combine two
   behind, and the drain interleaved dcomb/AV so the last chains hide.
 - Per row-tile: staged scores+exp / beta+D-combine+D^T / AV+scale+
   store, AV emitted at the TOP of each iteration so the PE never waits
   on the D chain; D-combine is a tensor_scalar (4x DVE mode) + in-place
   tensor_tensor add (2x mode) + one dma transpose -- the fused
   scalar_tensor_tensor only ran at 1x;
   small tiles get narrow dedicated D^T tiles so all eight stay
   resident without blowing SBUF; per-tile alpha in a persistent tile.
 - PE p-state warmup: dummy matmuls during the input-DMA window bring
   the PE to full clock (2.4GHz) before the first projection; without
   them the first ~3us of real matmuls run at half rate, and every
   recovery after a DMA stall costs a fresh ramp.
 - Hazards learned on hardware: dma_start_transpose is only safe on the
   sync queue (scalar-queue transposes intermittently corrupt data);
   fp8/DoubleRow matmuls are blocked by the 2e-2 accuracy budget.
"""

import numpy as np
import ml_dtypes
from contextlib import ExitStack

import concourse.bass as bass
import concourse.tile as tile
from concourse import bacc, mybir

B, T, C, HS, NT = 8, 2048, 1024, 128, 2
E2 = 2 * HS  # v/out feature dim (256)
THETA = 10000.0
NEG = -30.0
BF16, F32 = mybir.dt.bfloat16, mybir.dt.float32
AF = mybir.ActivationFunctionType
ALU = mybir.AluOpType
NCC = C // 128         # 8 contraction chunks
NTILE = T // 128       # 16 row tiles


def build_nc():
    nc = bacc.Bacc("TRN2", target_bir_lowering=False, debug=False, num_devices=8)

    xT = nc.declare_dram_parameter("xT", [C, T], BF16, isOutput=False)
    wqk = nc.declare_dram_parameter("wqk", [C, 4 * HS], BF16, isOutput=False)
    wv = nc.declare_dram_parameter("wv", [C, E2], BF16, isOutput=False)
    cosb = nc.declare_dram_parameter("cosb", [HS, T], BF16, isOutput=False)
    sinb = nc.declare_dram_parameter("sinb", [HS, T], BF16, isOutput=False)
    cmask = nc.declare_dram_parameter("cmask", [HS, 2 * HS], BF16, isOutput=False)
    lamc = nc.declare_dram_parameter("lamc", [HS, 2], F32, isOutput=False)
    outp = nc.declare_dram_parameter("out", [T, E2], BF16, isOutput=True)

    with tile.TileContext(nc) as tc:
        with ExitStack() as ctx:
            pers = ctx.enter_context(tc.tile_pool(name="pers", bufs=1))
            # psA: v-proj + AV accum ([128,512] f32 = 1 bank x 2)
            psA = ctx.enter_context(
                tc.tile_pool(name="psA", bufs=2, space="PSUM")
            )
            # psB: qk-proj groups + score chunks ([128,1024] f32 = 2 banks x 3)
            psB = ctx.enter_context(
                tc.tile_pool(name="psB", bufs=3, space="PSUM")
            )
            rp = ctx.enter_context(tc.tile_pool(name="rope", bufs=3))
            ep = ctx.enter_context(tc.tile_pool(name="ep", bufs=4))
            dp = ctx.enter_context(tc.tile_pool(name="dp", bufs=2))
            dtp = ctx.enter_context(tc.tile_pool(name="dtp", bufs=4))
            st = ctx.enter_context(tc.tile_pool(name="st", bufs=32))
            op = ctx.enter_context(tc.tile_pool(name="op", bufs=12))

            wqk_s = pers.tile([128, NCC * 4 * HS], BF16)  # chunk c at 512c
            wv_s = pers.tile([128, NCC * E2], BF16)       # chunk c at 256c
            cos_s = pers.tile([128, T], BF16)
            sin_s = pers.tile([128, T], BF16)
            msk_s = pers.tile([128, 2 * HS], BF16)        # [I | -30*triu]
            lam_s = pers.tile([128, 2], F32)              # [c0, c1/c0]
            alp_s = pers.tile([128, NTILE], F32)          # per-tile alpha
            xt_s = pers.tile([128, NCC, T], BF16, name="xt")
            # q/k tensors, tau: 0=q0 1=k0 2=q1 3=k1 (post-RoPE, [d', t])
            q_t = [
                pers.tile([128, T], BF16, name=f"q{t}", tag=f"q{t}")
                for t in range(4)
            ]
            v_t = [
                pers.tile([128, E2], BF16, name=f"v{j}", tag=f"v{j}")
                for j in range(NTILE)
            ]

            # ---- input DMAs, interleaved across four hwdge queues ----
            # consumption order for the first projection group is CORD; the
            # w chunk and x chunk for each c land together, round-robin over
            # queues so each queue moves ~1MB and x is fully resident ~4x
            # sooner than the v3 two-queue split.
            CORD = [3, 4, 0, 5, 1, 6, 2, 7]
            wqk_r = wqk[:].rearrange("(c p) w -> p c w", c=NCC)
            wqk_v = wqk_s[:].rearrange("p (c w) -> p c w", c=NCC)
            # The two hw queues alternate in CORD consumption order, each
            # pairing (w chunk c, x chunk c) so the first projection matmul
            # can start ~2.5us after queue start; cos/sin (needed by the
            # first RoPE) and wv (needed ~35us in) ride the gpsimd swdge.
            nc.gpsimd.dma_start(cos_s[:], cosb[:])
            nc.gpsimd.dma_start(sin_s[:], sinb[:])
            for ci, c in enumerate(CORD):
                q = nc.sync if ci % 2 == 0 else nc.scalar
                q.dma_start(wqk_v[:, c : c + 1, :], wqk_r[:, c : c + 1, :])
                q.dma_start(xt_s[:, c, 0:1024], xT[128 * c : 128 * (c + 1), 0:1024])
            for ci, c in enumerate(CORD):
                q = nc.sync if ci % 2 == 0 else nc.scalar
                q.dma_start(xt_s[:, c, 1024:2048], xT[128 * c : 128 * (c + 1), 1024:2048])
            nc.sync.dma_start(msk_s[:], cmask[:])
            nc.sync.dma_start(lam_s[:], lamc[:])
            nc.gpsimd.dma_start(wv_s[:].rearrange("p (c w) -> p c w", c=NCC),
                                wv[:].rearrange("(c p) w -> p c w", c=NCC))
            i_ap = msk_s[:, 0:128]
            u_ap = msk_s[:, 128:256]

            # ---- PE p-state warmup ----
            # The PE runs at ~1.2GHz until it has executed continuously for
            # ~3us, then 2.4GHz.  It would otherwise sit idle for ~7us of
            # input DMA after the preamble and start the projections cold;
            # dummy matmuls on a memset tile keep it busy so the real work
            # begins at full clock.
            warm = pers.tile([128, 512], BF16, name="warm")
            nc.vector.memset(warm[:], 0.0)
            pw = psA.tile([128, 512], F32, tag="av")

            def warm_mm(n):
                for _ in range(n):
                    nc.tensor.matmul(
                        pw[:],
                        warm[:, 0:128],
                        warm[:],
                        start=True,
                        stop=True,
                        skip_group_check=True,
                    )

            warm_mm(14)

            # ---- q/k projection + RoPE, per (tau, 1024-col group) ----
            # consume contraction chunks in DMA-arrival order; matmul
            # accumulation over c is commutative.
            def proj_qk(tau, g, gi):
                pj = psB.tile([128, 1024], F32, tag="sp")
                for ci, c in enumerate(CORD):
                    w_ap = wqk_s[:, 512 * c + 128 * tau : 512 * c + 128 * (tau + 1)]
                    nc.tensor.matmul(
                        pj[:, 0:512],
                        w_ap,
                        xt_s[:, c, 1024 * g : 1024 * g + 512],
                        start=(ci == 0),
                        stop=(ci == NCC - 1),
                        skip_group_check=True,
                    )
                    nc.tensor.matmul(
                        pj[:, 512:1024],
                        w_ap,
                        xt_s[:, c, 1024 * g + 512 : 1024 * (g + 1)],
                        start=(ci == 0),
                        stop=(ci == NCC - 1),
                        skip_group_check=True,
                    )
                sl = slice(1024 * g, 1024 * (g + 1))
                t1 = rp.tile([128, 1024], BF16, tag="t1")
                nc.vector.tensor_mul(t1[:], pj[:], cos_s[:, sl])
                u = rp.tile([128, 1024], BF16, tag="u")
                nc.vector.tensor_mul(u[:], pj[:], sin_s[:, sl])
                usw = rp.tile([128, 1024], BF16, tag="usw")
                # partition swap on the two hw queues (both idle during the
                # projection phase; the swdge adds ~2us of latency here and
                # gated the whole scores phase)
                qsw = nc.sync if gi % 2 == 0 else nc.scalar
                qsw.dma_start(usw[0:64, :], u[64:128, :])
                qsw.dma_start(usw[64:128, :], u[0:64, :])
                return (q_t[tau][:, sl], t1, usw)

            # The RoPE add for group gi is emitted during group gi+1's
            # matmuls so the partition-swap DMA latency never blocks the
            # DVE FIFO.  g=0 groups all come first: the 8 small row-tiles
            # (W <= 1024) depend only on the g=0 halves of q/k, so their
            # whole score->exp->D->D^T chains run DURING the g=1
            # projections (ACT/DVE/sync are otherwise idle there).
            GORD0 = [(0, 0), (0, 1), (0, 2), (0, 3)]
            GORD1 = [(1, 0), (1, 1), (1, 2), (1, 3)]
            pend = None
            for gi, (g, tau) in enumerate(GORD0):
                nxt = proj_qk(tau, g, gi)
                if pend is not None:
                    nc.vector.tensor_add(pend[0], pend[1][:], pend[2][:])
                pend = nxt

            # ---- v projection for s-block j ----
            def proj_v(j):
                vp = psA.tile([128, 512], F32, tag="av")
                for c in range(NCC):
                    nc.tensor.matmul(
                        vp[:, :E2],
                        xt_s[:, c, 128 * j : 128 * (j + 1)],
                        wv_s[:, E2 * c : E2 * (c + 1)],
                        start=(c == 0),
                        stop=(c == NCC - 1),
                        skip_group_check=True,
                    )
                nc.vector.tensor_copy(v_t[j][:], vp[:, :E2])

            # ---- staged per-row-tile pipeline ----
            # stage S: score matmuls + exp (PE -> ACT)
            # stage D: beta, D-combine, D^T transpose (DVE -> sync queue)
            # stage A: AV matmuls + out scale + store (PE -> DVE -> gpsimd)
            # D lags S by 1 iteration and A lags S by LAG so the
            # exp -> beta -> D -> D^T -> sem chain is never on the PE
            # critical path.
            parts = {}
            dts = {}

            def scores_part(i, pi):
                W = 128 * (i + 1)
                nch = (W + 1023) // 1024
                es, ls = [], []
                for n in range(2):
                    en = ep.tile([128, T], BF16, tag=f"E{n}")
                    lp = st.tile([128, 2], F32, tag=f"lp{n}")
                    for ch in range(nch):
                        off = 1024 * ch
                        wch = min(1024, W - off)
                        sp = psB.tile([128, 1024], F32, tag="sp")
                        for sub in range(0, wch, 512):
                            wsub = min(512, wch - sub)
                            diag = off + sub + wsub == W
                            nc.tensor.matmul(
                                sp[:, sub : sub + wsub],
                                q_t[2 * n][:, 128 * i : 128 * (i + 1)],
                                q_t[2 * n + 1][:, off + sub : off + sub + wsub],
                                start=True,
                                stop=not diag,
                                skip_group_check=True,
                            )
                            if diag:
                                nc.tensor.matmul(
                                    sp[:, sub + wsub - 128 : sub + wsub],
                                    i_ap,
                                    u_ap,
                                    start=False,
                                    stop=True,
                                    skip_group_check=True,
                                )
                        nc.scalar.activation(
                            en[:, off : off + wch],
                            sp[:, :wch],
                            AF.Exp,
                            accum_out=lp[:, ch : ch + 1],
                        )
                    es.append(en)
                    ls.append(lp)
                parts[i] = (es, ls, nch)

            def dcomb_part(i):
                es, lps, nch = parts.pop(i)
                W = 128 * (i + 1)
                ls = []
                for n in range(2):
                    if nch == 1:
                        ls.append(lps[n][:, 0:1])
                    else:
                        ln = st.tile([128, 1], F32, tag=f"l{n}")
                        nc.vector.tensor_add(ln[:], lps[n][:, 0:1], lps[n][:, 1:2])
                        ls.append(ln[:])
                r1 = st.tile([128, 1], F32, tag="r1")
                nc.vector.reciprocal(r1[:], ls[1])
                beta = st.tile([128, 1], F32, tag="beta")
                nc.vector.tensor_scalar(
                    beta[:], ls[0], r1[:], lam_s[:, 1:2], ALU.mult, ALU.mult
                )

                d = dp.tile([128, T], BF16, tag="d")
                if i <= 7:
                    # small tiles: narrow dedicated dt (kept alive across the
                    # whole phase-3 prologue without blowing SBUF)
                    dt = dtp.tile([128, i + 1, 128], BF16, tag=f"dts{i}",
                                  bufs=1, name=f"dts{i}")
                else:
                    dt = dtp.tile([128, NTILE, 128], BF16, tag="dt")
                # two-op D-combine: tensor_scalar gets the 4x DVE mode and
                # tensor_tensor the 2x mode (bf16 SBUF), vs 1x for the
                # single fused scalar_tensor_tensor
                nc.vector.tensor_scalar(
                    d[:, 0:W], es[1][:, 0:W], beta[:], None, ALU.mult
                )
                nc.vector.tensor_add(d[:, 0:W], d[:, 0:W], es[0][:, 0:W])
                # NOTE: dma_start_transpose must stay on the sync queue --
                # on the scalar (ACT) queue it intermittently corrupts data
                # (observed rel-err jumps 0.006 -> 0.014..0.031 on hardware).
                nc.sync.dma_start_transpose(dt[:, 0 : W // 128, :], d[:, 0:W])
                dts[i] = dt
                # alpha is only needed by the AV scale several iterations
                # later; computing it after the D chain shortens the
                # beta->stt critical path
                r0 = st.tile([128, 1], F32, tag="r0")
                nc.vector.reciprocal(r0[:], ls[0])
                nc.vector.tensor_mul(alp_s[:, i : i + 1], r0[:], lam_s[:, 0:1])

            # ---- AV + out for row-tile i ----
            # tail=True routes the scale to ACT and the store to the scalar
            # queue: in the drain phase the DVE/sync FIFOs are clogged with
            # the last D-combines/transposes while ACT+scalar sit idle.
            def av_part(i, pi, tail=False):
                dt = dts.pop(i)
                av = psA.tile([128, 512], F32, tag="av")
                for j in range(i + 1):
                    nc.tensor.matmul(
                        av[:, :E2],
                        dt[:, j, :],
                        v_t[j][:],
                        start=(j == 0),
                        stop=(j == i),
                        skip_group_check=True,
                    )
                ot = op.tile([128, E2], BF16, tag="ot")
                if tail:
                    nc.scalar.activation(
                        ot[:], av[:, :E2], AF.Copy,
                        scale=alp_s[:, i : i + 1],
                    )
                    nc.scalar.dma_start(outp[128 * i : 128 * (i + 1), :], ot[:])
                else:
                    nc.vector.tensor_scalar(
                        ot[:], av[:, :E2], alp_s[:, i : i + 1], None, ALU.mult
                    )
                    nc.sync.dma_start(outp[128 * i : 128 * (i + 1), :], ot[:])

            # AV runs two row-tiles behind scores so the PE never waits on
            # the exp -> beta -> D -> D^T chain.  v-projections are packed
            # into the early (small) iterations as PE filler.  The two
            # smallest row-tiles go LAST so the exposed end-of-loop chain
            # belongs to tiny tiles.
            # ---- phase 2: g=1 projections + v-projs 0..7 + all eight
            # small tiles' score chains ----
            for k, (g, tau) in enumerate(GORD1):
                nxt = proj_qk(tau, g, 4 + k)
                nc.vector.tensor_add(pend[0], pend[1][:], pend[2][:])
                pend = nxt
                proj_v(2 * k)
                proj_v(2 * k + 1)
                scores_part(2 * k, k)
                scores_part(2 * k + 1, k)
                if k >= 1:
                    dcomb_part(2 * k - 2)
                    dcomb_part(2 * k - 1)
            nc.vector.tensor_add(pend[0], pend[1][:], pend[2][:])
            dcomb_part(6)
            dcomb_part(7)

            # ---- phase 3: big tiles descending; small AVs are the early
            # PE filler, the tail ends on the smallest big tiles ----
            ORDER3 = [15, 14, 13, 12, 11, 10, 9, 8]
            for p, i in enumerate(ORDER3):
                if p <= 3:
                    av_part(2 * p, 8 + p)
                    av_part(2 * p + 1, 8 + p)
                    proj_v(8 + 2 * p)
                    proj_v(9 + 2 * p)
                else:
                    av_part(ORDER3[p - 4], 8 + p)
                scores_part(i, 8 + p)
                if p >= 2:
                    dcomb_part(ORDER3[p - 2])
            dcomb_part(9)
            av_part(11, 16)
            dcomb_part(8)
            av_part(10, 17)
            av_part(9, 18)
            av_part(8, 19)

    nc.compile()
    return nc


_CACHE = {}


def _get_nc():
    if "nc" not in _CACHE:
        _CACHE["nc"] = build_nc()
    return _CACHE["nc"]


def _prep_host(x, Wq, Wk, Wv, lambda_q, lambda_k, layer_idx):
    bf = ml_dtypes.bfloat16
    perm = np.concatenate([np.arange(0, HS, 2), np.arange(1, HS, 2)])
    scale = 1.0 / np.sqrt(HS)
    Wqp = np.asarray(Wq, np.float32)[:, :, perm] * scale
    Wkp = np.asarray(Wk, np.float32)[:, :, perm]
    # tau order: q0, k0, q1, k1
    wqk = np.concatenate([Wqp[0], Wkp[0], Wqp[1], Wkp[1]], axis=1).astype(bf)
    wv = np.asarray(Wv, np.float32).astype(bf)

    f = 1.0 / THETA ** (np.arange(0, HS, 2, dtype=np.float64) / HS)
    ang = np.outer(f, np.arange(T, dtype=np.float64))  # [64, T]
    cosb = np.concatenate([np.cos(ang), np.cos(ang)], 0).astype(bf)
    # swap-after-mul RoPE: u = qb * sinb; usw = partition-swap(u);
    # out = qb*cos + usw.  Want usw[0:64] = -sin*xi, usw[64:128] = +sin*xr
    # => sinb rows = [+sin ; -sin]
    sinb = np.concatenate([np.sin(ang), -np.sin(ang)], 0).astype(bf)

    eye = np.eye(HS, dtype=np.float32)
    u30 = NEG * np.triu(np.ones((HS, HS), np.float32), k=1)
    cmask = np.concatenate([eye, u30], axis=1).astype(bf)

    li = float(np.asarray(layer_idx))
    lam_init = 0.8 - 0.6 * np.exp(-0.3 * (li - 1.0))
    e = np.mean(
        np.exp(np.asarray(lambda_q, np.float32) * np.asarray(lambda_k, np.float32)),
        axis=-1,
    )
    lam = e - np.concatenate([[0.0], e[:-1]]) + lam_init
    c0, c1 = float(lam[0]), float(-lam[1])
    lamc = np.tile(np.array([[c0, c1 / c0]], np.float32), (HS, 1))

    xT = np.ascontiguousarray(np.asarray(x, np.float32).transpose(0, 2, 1)).astype(bf)
    return xT, wqk, wv, cosb, sinb, cmask, lamc


def _make_in_maps(np_inputs):
    xT, wqk, wv, cosb, sinb, cmask, lamc = _prep_host(**np_inputs)
    return [
        {
            "xT": xT[b],
            "wqk": wqk,
            "wv": wv,
            "cosb": cosb,
            "sinb": sinb,
            "cmask": cmask,
            "lamc": lamc,
        }
        for b in range(B)
    ]


def _collect(res):
    return np.stack([res.results[b]["out"] for b in range(B)]).astype(np.float32)


def kernel(x, Wq, Wk, Wv, lambda_q, lambda_k, layer_idx):
    from concourse.bass_utils import run_bass_kernel_spmd

    in_maps = _make_in_maps(
        dict(x=x, Wq=Wq, Wk=Wk, Wv=Wv, lambda_q=lambda_q,
             lambda_k=lambda_k, layer_idx=layer_idx)
    )
    res = run_bass_kernel_spmd(_get_nc(), in_maps, core_ids=list(range(B)))
    return _collect(res)


# revision 71
# speedup vs baseline: 1.0384x; 1.0384x over previous
"""AlternatingDiffHead Trainium2 kernel.

Data-parallel over batch: B=8 batch elements -> 8 NeuronCores, one batch
element per core, no collectives.

Per-core math (T=2048, C=1024, HS=128, 2 terms):
  v  = x @ Wv                                  [T, 256]
  qn = rope(x @ Wqn * 1/sqrt(HS)),  kn = rope(x @ Wkn)     [T, 128]
  Sn = qn @ kn^T  (causal)                      [T, T]
  En = exp(Sn)    (no max-sub; S is O(1))       rowsum -> ln
  D  = E0 + beta E1,  beta[t] = (c1 l0[t]) / (c0 l1[t])
  out[t] = (c0 / l0[t]) * (D @ v)[t]
where c0 = lam0, c1 = -lam1 (host-computed scalars).

v6 design notes (on top of v3):
 - Inputs interleaved (w chunk c, x chunk-half c) across both hw queues
   in consumption order, x in g-halves matching the projection loop, so
   the first matmul starts ~14us in and is never starved.
 - RoPE add for group g emitted during group g+1's matmuls so the
   partition-swap DMA latency never blocks the DVE FIFO (this gated the
   whole scores phase by ~15us).  Swaps ride the idle hw queues.
 - Three phases: (1) g=0 projections; (2) g=1 projections overlapped
   with v-projs 0..7 and the complete score->exp->D->D^T chains of the
   eight small row-tiles (W<=1024 needs only the g=0 q/k halves) --
   ACT/DVE/sync work there instead of idling until all projections
   finish; (3) big tiles descending [15..8] with the small AVs as early
   PE filler, AV(i) four iterations behind scores(i), D-combine two
   behind, and the drain interleaved dcomb/AV so the last chains hide.
 - Per row-tile: staged scores+exp / beta+D-combine+D^T / AV+scale+
   store, AV emitted at the TOP of each iteration so the PE never waits
   on the D chain; D-combine is a tensor_scalar (4x DVE mode) + in-place
   tensor_tensor add (2x mode) + one dma transpose -- the fused
   scalar_tensor_tensor only ran at 1x;
   small tiles get narrow dedicated D^T tiles so all eight stay
   resident without blowing SBUF; per-tile alpha in a persistent tile.
 - PE p-state warmup: dummy matmuls during the input-DMA window bring
   the PE to full clock (2.4GHz) before the first projection; without
   them the first ~3us of real matmuls run at half rate, and every
   recovery after a DMA stall costs a fresh ramp.
 - Hazards learned on hardware: dma_start_transpose is only safe on the
   sync queue (scalar-queue transposes intermittently corrupt data);
   fp8/DoubleRow matmuls are blocked by the 2e-2 accuracy budget.
"""

import numpy as np
import ml_dtypes
from contextlib import ExitStack

import concourse.bass as bass
import concourse.tile as tile
from concourse import bacc, mybir

B, T, C, HS, NT = 8, 2048, 1024, 128, 2
E2 = 2 * HS  # v/out feature dim (256)
THETA = 10000.0
NEG = -30.0
BF16, F32 = mybir.dt.bfloat16, mybir.dt.float32
AF = mybir.ActivationFunctionType
ALU = mybir.AluOpType
NCC = C // 128         # 8 contraction chunks
NTILE = T // 128       # 16 row tiles


def build_nc():
    nc = bacc.Bacc("TRN2", target_bir_lowering=False, debug=False, num_devices=8)

    xT = nc.declare_dram_parameter("xT", [C, T], BF16, isOutput=False)
    wqk = nc.declare_dram_parameter("wqk", [C, 4 * HS], BF16, isOutput=False)
    wv = nc.declare_dram_parameter("wv", [C, E2], BF16, isOutput=False)
    cosb = nc.declare_dram_parameter("cosb", [HS, T], BF16, isOutput=False)
    sinb = nc.declare_dram_parameter("sinb", [HS, T], BF16, isOutput=False)
    cmask = nc.declare_dram_parameter("cmask", [HS, 2 * HS], BF16, isOutput=False)
    lamc = nc.declare_dram_parameter("lamc", [HS, 2], F32, isOutput=False)
    outp = nc.declare_dram_parameter("out", [T, E2], BF16, isOutput=True)

    with tile.TileContext(nc) as tc:
        with ExitStack() as ctx:
            pers = ctx.enter_context(tc.tile_pool(name="pers", bufs=1))
            # psA: v-proj + AV accum ([128,512] f32 = 1 bank x 2)
            psA = ctx.enter_context(
                tc.tile_pool(name="psA", bufs=2, space="PSUM")
            )
            # psB: qk-proj groups + score chunks ([128,1024] f32 = 2 banks x 3)
            psB = ctx.enter_context(
                tc.tile_pool(name="psB", bufs=3, space="PSUM")
            )
            rp = ctx.enter_context(tc.tile_pool(name="rope", bufs=3))
            ep = ctx.enter_context(tc.tile_pool(name="ep", bufs=4))
            dp = ctx.enter_context(tc.tile_pool(name="dp", bufs=2))
            dtp = ctx.enter_context(tc.tile_pool(name="dtp", bufs=4))
            st = ctx.enter_context(tc.tile_pool(name="st", bufs=32))
            op = ctx.enter_context(tc.tile_pool(name="op", bufs=12))

            wqk_s = pers.tile([128, NCC * 4 * HS], BF16)  # chunk c at 512c
            wv_s = pers.tile([128, NCC * E2], BF16)       # chunk c at 256c
            cos_s = pers.tile([128, T], BF16)
            sin_s = pers.tile([128, T], BF16)
            msk_s = pers.tile([128, 2 * HS], BF16)        # [I | -30*triu]
            lam_s = pers.tile([128, 2], F32)              # [c0, c1/c0]
            alp_s = pers.tile([128, NTILE], F32)          # per-tile alpha
            xt_s = pers.tile([128, NCC, T], BF16, name="xt")
            # q/k tensors, tau: 0=q0 1=k0 2=q1 3=k1 (post-RoPE, [d', t])
            q_t = [
                pers.tile([128, T], BF16, name=f"q{t}", tag=f"q{t}")
                for t in range(4)
            ]
            v_t = [
                pers.tile([128, E2], BF16, name=f"v{j}", tag=f"v{j}")
                for j in range(NTILE)
            ]

            # ---- input DMAs, interleaved across four hwdge queues ----
            # consumption order for the first projection group is CORD; the
            # w chunk and x chunk for each c land together, round-robin over
            # queues so each queue moves ~1MB and x is fully resident ~4x
            # sooner than the v3 two-queue split.
            CORD = [3, 4, 0, 5, 1, 6, 2, 7]
            wqk_r = wqk[:].rearrange("(c p) w -> p c w", c=NCC)
            wqk_v = wqk_s[:].rearrange("p (c w) -> p c w", c=NCC)
            # The two hw queues alternate in CORD consumption order, each
            # pairing (w chunk c, x chunk c) so the first projection matmul
            # can start ~2.5us after queue start; cos/sin (needed by the
            # first RoPE) and wv (needed ~35us in) ride the gpsimd swdge.
            nc.gpsimd.dma_start(cos_s[:], cosb[:])
            nc.gpsimd.dma_start(sin_s[:], sinb[:])
            for ci, c in enumerate(CORD):
                q = nc.sync if ci % 2 == 0 else nc.scalar
                q.dma_start(wqk_v[:, c : c + 1, :], wqk_r[:, c : c + 1, :])
                q.dma_start(xt_s[:, c, 0:1024], xT[128 * c : 128 * (c + 1), 0:1024])
            for ci, c in enumerate(CORD):
                q = nc.sync if ci % 2 == 0 else nc.scalar
                q.dma_start(xt_s[:, c, 1024:2048], xT[128 * c : 128 * (c + 1), 1024:2048])
            nc.sync.dma_start(msk_s[:], cmask[:])
            nc.sync.dma_start(lam_s[:], lamc[:])
            nc.gpsimd.dma_start(wv_s[:].rearrange("p (c w) -> p c w", c=NCC),
                                wv[:].rearrange("(c p) w -> p c w", c=NCC))
            i_ap = msk_s[:, 0:128]
            u_ap = msk_s[:, 128:256]

            # ---- PE p-state warmup ----
            # The PE runs at ~1.2GHz until it has executed continuously for
            # ~3us, then 2.4GHz.  It would otherwise sit idle for ~7us of
            # input DMA after the preamble and start the projections cold;
            # dummy matmuls on a memset tile keep it busy so the real work
            # begins at full clock.
            warm = pers.tile([128, 512], BF16, name="warm")
            nc.vector.memset(warm[:], 0.0)
            pw = psA.tile([128, 512], F32, tag="av")

            def warm_mm(n):
                for _ in range(n):
                    nc.tensor.matmul(
                        pw[:],
                        warm[:, 0:128],
                        warm[:],
                        start=True,
                        stop=True,
                        skip_group_check=True,
                    )

            warm_mm(14)

            # ---- q/k projection + RoPE, per (tau, 1024-col group) ----
            # consume contraction chunks in DMA-arrival order; matmul
            # accumulation over c is commutative.
            def proj_qk(tau, g, gi):
                pj = psB.tile([128, 1024], F32, tag="sp")
                for ci, c in enumerate(CORD):
                    w_ap = wqk_s[:, 512 * c + 128 * tau : 512 * c + 128 * (tau + 1)]
                    nc.tensor.matmul(
                        pj[:, 0:512],
                        w_ap,
                        xt_s[:, c, 1024 * g : 1024 * g + 512],
                        start=(ci == 0),
                        stop=(ci == NCC - 1),
                        skip_group_check=True,
                    )
                    nc.tensor.matmul(
                        pj[:, 512:1024],
                        w_ap,
                        xt_s[:, c, 1024 * g + 512 : 1024 * (g + 1)],
                        start=(ci == 0),
                        stop=(ci == NCC - 1),
                        skip_group_check=True,
                    )
                sl = slice(1024 * g, 1024 * (g + 1))
                t1 = rp.tile([128, 1024], BF16, tag="t1")
                nc.vector.tensor_mul(t1[:], pj[:], cos_s[:, sl])
                u = rp.tile([128, 1024], BF16, tag="u")
                nc.vector.tensor_mul(u[:], pj[:], sin_s[:, sl])
                usw = rp.tile([128, 1024], BF16, tag="usw")
                # partition swap on the two hw queues (both idle during the
                # projection phase; the swdge adds ~2us of latency here and
                # gated the whole scores phase)
                qsw = nc.sync if gi % 2 == 0 else nc.scalar
                qsw.dma_start(usw[0:64, :], u[64:128, :])
                qsw.dma_start(usw[64:128, :], u[0:64, :])
                return (q_t[tau][:, sl], t1, usw)

            # The RoPE add for group gi is emitted during group gi+1's
            # matmuls so the partition-swap DMA latency never blocks the
            # DVE FIFO.  g=0 groups all come first: the 8 small row-tiles
            # (W <= 1024) depend only on the g=0 halves of q/k, so their
            # whole score->exp->D->D^T chains run DURING the g=1
            # projections (ACT/DVE/sync are otherwise idle there).
            GORD0 = [(0, 0), (0, 1), (0, 2), (0, 3)]
            GORD1 = [(1, 0), (1, 1), (1, 2), (1, 3)]
            pend = None
            for gi, (g, tau) in enumerate(GORD0):
                nxt = proj_qk(tau, g, gi)
                if pend is not None:
                    nc.vector.tensor_add(pend[0], pend[1][:], pend[2][:])
                pend = nxt

            # ---- v projection for s-block j ----
            def proj_v(j):
                vp = psA.tile([128, 512], F32, tag="av")
                for c in range(NCC):
                    nc.tensor.matmul(
                        vp[:, :E2],
                        xt_s[:, c, 128 * j : 128 * (j + 1)],
                        wv_s[:, E2 * c : E2 * (c + 1)],
                        start=(c == 0),
                        stop=(c == NCC - 1),
                        skip_group_check=True,
                    )
                nc.vector.tensor_copy(v_t[j][:], vp[:, :E2])

            # ---- staged per-row-tile pipeline ----
            # stage S: score matmuls + exp (PE -> ACT)
            # stage D: beta, D-combine, D^T transpose (DVE -> sync queue)
            # stage A: AV matmuls + out scale + store (PE -> DVE -> gpsimd)
            # D lags S by 1 iteration and A lags S by LAG so the
            # exp -> beta -> D -> D^T -> sem chain is never on the PE
            # critical path.
            parts = {}
            dts = {}

            def scores_part(i, pi):
                W = 128 * (i + 1)
                nch = (W + 1023) // 1024
                es, ls = [], []
                for n in range(2):
                    en = ep.tile([128, T], BF16, tag=f"E{n}")
                    lp = st.tile([128, 2], F32, tag=f"lp{n}")
                    for ch in range(nch):
                        off = 1024 * ch
                        wch = min(1024, W - off)
                        sp = psB.tile([128, 1024], F32, tag="sp")
                        for sub in range(0, wch, 512):
                            wsub = min(512, wch - sub)
                            diag = off + sub + wsub == W
                            nc.tensor.matmul(
                                sp[:, sub : sub + wsub],
                                q_t[2 * n][:, 128 * i : 128 * (i + 1)],
                                q_t[2 * n + 1][:, off + sub : off + sub + wsub],
                                start=True,
                                stop=not diag,
                                skip_group_check=True,
                            )
                            if diag:
                                nc.tensor.matmul(
                                    sp[:, sub + wsub - 128 : sub + wsub],
                                    i_ap,
                                    u_ap,
                                    start=False,
                                    stop=True,
                                    skip_group_check=True,
                                )
                        nc.scalar.activation(
                            en[:, off : off + wch],
                            sp[:, :wch],
                            AF.Exp,
                            accum_out=lp[:, ch : ch + 1],
                        )
                    es.append(en)
                    ls.append(lp)
                parts[i] = (es, ls, nch)

            def dcomb_part(i):
                es, lps, nch = parts.pop(i)
                W = 128 * (i + 1)
                ls = []
                for n in range(2):
                    if nch == 1:
                        ls.append(lps[n][:, 0:1])
                    else:
                        ln = st.tile([128, 1], F32, tag=f"l{n}")
                        nc.vector.tensor_add(ln[:], lps[n][:, 0:1], lps[n][:, 1:2])
                        ls.append(ln[:])
                r1 = st.tile([128, 1], F32, tag="r1")
                nc.vector.reciprocal(r1[:], ls[1])
                beta = st.tile([128, 1], F32, tag="beta")
                nc.vector.tensor_scalar(
                    beta[:], ls[0], r1[:], lam_s[:, 1:2], ALU.mult, ALU.mult
                )

                d = dp.tile([128, T], BF16, tag="d")
                if i <= 7:
                    # small tiles: narrow dedicated dt (kept alive across the
                    # whole phase-3 prologue without blowing SBUF)
                    dt = dtp.tile([128, i + 1, 128], BF16, tag=f"dts{i}",
                                  bufs=1, name=f"dts{i}")
                else:
                    dt = dtp.tile([128, NTILE, 128], BF16, tag="dt")
                # two-op D-combine: tensor_scalar gets the 4x DVE mode and
                # tensor_tensor the 2x mode (bf16 SBUF), vs 1x for the
                # single fused scalar_tensor_tensor
                nc.vector.tensor_scalar(
                    d[:, 0:W], es[1][:, 0:W], beta[:], None, ALU.mult
                )
                nc.vector.tensor_add(d[:, 0:W], d[:, 0:W], es[0][:, 0:W])
                # NOTE: dma_start_transpose must stay on the sync queue --
                # on the scalar (ACT) queue it intermittently corrupts data
                # (observed rel-err jumps 0.006 -> 0.014..0.031 on hardware).
                nc.sync.dma_start_transpose(dt[:, 0 : W // 128, :], d[:, 0:W])
                dts[i] = dt
                # alpha is only needed by the AV scale several iterations
                # later; computing it after the D chain shortens the
                # beta->stt critical path
                r0 = st.tile([128, 1], F32, tag="r0")
                nc.vector.reciprocal(r0[:], ls[0])
                nc.vector.tensor_mul(alp_s[:, i : i + 1], r0[:], lam_s[:, 0:1])

            # ---- AV + out for row-tile i ----
            # tail=True routes the scale to ACT and the store to the scalar
            # queue: in the drain phase the DVE/sync FIFOs are clogged with
            # the last D-combines/transposes while ACT+scalar sit idle.
            def av_part(i, pi, tail=False):
                dt = dts.pop(i)
                av = psA.tile([128, 512], F32, tag="av")
                for j in range(i + 1):
                    nc.tensor.matmul(
                        av[:, :E2],
                        dt[:, j, :],
                        v_t[j][:],
                        start=(j == 0),
                        stop=(j == i),
                        skip_group_check=True,
                    )
                ot = op.tile([128, E2], BF16, tag="ot")
                if tail:
                    nc.scalar.activation(
                        ot[:], av[:, :E2], AF.Copy,
                        scale=alp_s[:, i : i + 1],
                    )
                    nc.scalar.dma_start(outp[128 * i : 128 * (i + 1), :], ot[:])
                else:
                    nc.vector.tensor_scalar(
                        ot[:], av[:, :E2], alp_s[:, i : i + 1], None, ALU.mult
                    )
                    nc.sync.dma_start(outp[128 * i : 128 * (i + 1), :], ot[:])

            # AV runs two row-tiles behind scores so the PE never waits on
            # the exp -> beta -> D -> D^T chain.  v-projections are packed
            # into the early (small) iterations as PE filler.  The two
            # smallest row-tiles go LAST so the exposed end-of-loop chain
            # belongs to tiny tiles.
            # ---- phase 2: g=1 projections + v-projs 0..7 + all eight
            # small tiles' score chains ----
            for k, (g, tau) in enumerate(GORD1):
                nxt = proj_qk(tau, g, 4 + k)
                nc.vector.tensor_add(pend[0], pend[1][:], pend[2][:])
                pend = nxt
                proj_v(2 * k)
                proj_v(2 * k + 1)
                scores_part(2 * k, k)
                scores_part(2 * k + 1, k)
                if k >= 1:
                    dcomb_part(2 * k - 2)
                    dcomb_part(2 * k - 1)
            nc.vector.tensor_add(pend[0], pend[1][:], pend[2][:])
            dcomb_part(6)
            dcomb_part(7)

            # ---- phase 3: big tiles descending; small AVs are the early
            # PE filler, the tail ends on the smallest big tiles ----
            # big tiles processed in PAIRS: the per-iteration cross-engine
            # chain latency (~1.8us of sem hops and queue latency) is paid
            # once per two tiles instead of per tile.
            PAIRS = [(15, 14), (13, 12), (11, 10), (9, 8)]
            for s, (ia, ib) in enumerate(PAIRS):
                if s <= 1:
                    for k in range(4):
                        av_part(4 * s + k, 8 + s)
                    for k in range(4):
                        proj_v(8 + 4 * s + k)
                else:
                    ja, jb = PAIRS[s - 2]
                    av_part(ja, 8 + s)
                    av_part(jb, 8 + s)
                scores_part(ia, 8 + s)
                scores_part(ib, 8 + s)
                if s >= 1:
                    ja, jb = PAIRS[s - 1]
                    dcomb_part(ja)
                    dcomb_part(jb)
            dcomb_part(9)
            av_part(11, 16)
            dcomb_part(8)
            av_part(10, 17)
            av_part(9, 18)
            av_part(8, 19)

    nc.compile()
    return nc


_CACHE = {}


def _get_nc():
    if "nc" not in _CACHE:
        _CACHE["nc"] = build_nc()
    return _CACHE["nc"]


def _prep_host(x, Wq, Wk, Wv, lambda_q, lambda_k, layer_idx):
    bf = ml_dtypes.bfloat16
    perm = np.concatenate([np.arange(0, HS, 2), np.arange(1, HS, 2)])
    scale = 1.0 / np.sqrt(HS)
    Wqp = np.asarray(Wq, np.float32)[:, :, perm] * scale
    Wkp = np.asarray(Wk, np.float32)[:, :, perm]
    # tau order: q0, k0, q1, k1
    wqk = np.concatenate([Wqp[0], Wkp[0], Wqp[1], Wkp[1]], axis=1).astype(bf)
    wv = np.asarray(Wv, np.float32).astype(bf)

    f = 1.0 / THETA ** (np.arange(0, HS, 2, dtype=np.float64) / HS)
    ang = np.outer(f, np.arange(T, dtype=np.float64))  # [64, T]
    cosb = np.concatenate([np.cos(ang), np.cos(ang)], 0).astype(bf)
    # swap-after-mul RoPE: u = qb * sinb; usw = partition-swap(u);
    # out = qb*cos + usw.  Want usw[0:64] = -sin*xi, usw[64:128] = +sin*xr
    # => sinb rows = [+sin ; -sin]
    sinb = np.concatenate([np.sin(ang), -np.sin(ang)], 0).astype(bf)

    eye = np.eye(HS, dtype=np.float32)
    u30 = NEG * np.triu(np.ones((HS, HS), np.float32), k=1)
    cmask = np.concatenate([eye, u30], axis=1).astype(bf)

    li = float(np.asarray(layer_idx))
    lam_init = 0.8 - 0.6 * np.exp(-0.3 * (li - 1.0))
    e = np.mean(
        np.exp(np.asarray(lambda_q, np.float32) * np.asarray(lambda_k, np.float32)),
        axis=-1,
    )
    lam = e - np.concatenate([[0.0], e[:-1]]) + lam_init
    c0, c1 = float(lam[0]), float(-lam[1])
    lamc = np.tile(np.array([[c0, c1 / c0]], np.float32), (HS, 1))

    xT = np.ascontiguousarray(np.asarray(x, np.float32).transpose(0, 2, 1)).astype(bf)
    return xT, wqk, wv, cosb, sinb, cmask, lamc


def _make_in_maps(np_inputs):
    xT, wqk, wv, cosb, sinb, cmask, lamc = _prep_host(**np_inputs)
    return [
        {
            "xT": xT[b],
            "wqk": wqk,
            "wv": wv,
            "cosb": cosb,
            "sinb": sinb,
            "cmask": cmask,
            "lamc": lamc,
        }
        for b in range(B)
    ]


def _collect(res):
    return np.stack([res.results[b]["out"] for b in range(B)]).astype(np.float32)


def kernel(x, Wq, Wk, Wv, lambda_q, lambda_k, layer_idx):
    from concourse.bass_utils import run_bass_kernel_spmd

    in_maps = _make_in_maps(
        dict(x=x, Wq=Wq, Wk=Wk, Wv=Wv, lambda_q=lambda_q,
             lambda_k=lambda_k, layer_idx=layer_idx)
    )
    res = run_bass_kernel_spmd(_get_nc(), in_maps, core_ids=list(range(B)))
    return _collect(res)
